# revision 32
# baseline (speedup 1.0000x reference)
"""Invariant Point Attention kernel for 8 Trainium2 NeuronCores.

Shapes (hardcoded): features [2, 1024, 512], H=8 heads, D=64, P=4 points.
Sharding: 8 cores = 2 batches x 4 head-pairs. Core c handles batch c//4 and
heads (2*(c%4), 2*(c%4)+1). Each core computes its heads' attention plus the
row-slice of the output projection; the host sums the 4 partials per batch.

Math notes:
  - softmax(logits) is invariant to per-row shifts, so the -pscale*|q_p|^2 term
    is dropped entirely; the -pscale*|k_p|^2 term enters as a per-partition
    bias on the Exp activation (logits are computed transposed, so k-index j
    is the partition dim there).
  - ln_g folds into the projection weights on the host; ln_b and projection
    biases fold into per-output-channel constants.
  - The softmax denominator comes from an appended ones-column in the V matrix
    of the attention*V matmul (row 64 of the accumulator = row sums).
  - Matmul operands are bitcast to float32r (full-rate fp32 PE mode).
  - Point q/k projections for both heads are fused into one M=128 stationary
    matrix, each 12-wide block padded to a 32-partition boundary so PSUM reads
    stay quadrant-aligned.
"""

import numpy as np

B, L, C = 2, 1024, 512
H, P = 8, 4
D = C // H  # 64
LN_EPS = 1e-5
NCORES = 8
HPC = 2  # heads per core

_CACHE = {}


def _build_program(use_bias: bool, use_bo: bool):
    import concourse.bacc as bacc
    import concourse.tile as tile
    import concourse.mybir as mybir
    from concourse.masks import make_identity

    f32 = mybir.dt.float32
    f32r = mybir.dt.float32r
    AF = mybir.ActivationFunctionType
    ALU = mybir.AluOpType

    nc = bacc.Bacc(None, target_bir_lowering=False)

    # ---- DRAM I/O (per-core contents supplied via in_maps) ----
    xb = nc.dram_tensor("xb", [L, C], f32, kind="ExternalInput")
    wq2 = nc.dram_tensor("wq2", [C, HPC * D], f32r, kind="ExternalInput")
    wk2 = nc.dram_tensor("wk2", [C, HPC * D], f32r, kind="ExternalInput")
    wv2 = nc.dram_tensor("wv2", [C, HPC * D], f32r, kind="ExternalInput")
    wpt = nc.dram_tensor("wpt", [C, 128], f32r, kind="ExternalInput")
    wo2 = nc.dram_tensor("wo2", [HPC * D, C], f32r, kind="ExternalInput")
    psc_pt = nc.dram_tensor("psc_pt", [128, 1], f32, kind="ExternalInput")
    psc_negcol = nc.dram_tensor("psc_negcol", [128, HPC], f32, kind="ExternalInput")
    if use_bias:
        bq2 = nc.dram_tensor("bq2", [HPC * D, 1], f32, kind="ExternalInput")
        bk2 = nc.dram_tensor("bk2", [HPC * D, 1], f32, kind="ExternalInput")
        bv2 = nc.dram_tensor("bv2", [HPC * D, 1], f32, kind="ExternalInput")
        b_pt = nc.dram_tensor("b_pt", [128, 1], f32, kind="ExternalInput")
    if use_bo:
        bo_d = nc.dram_tensor("bo_d", [1, C], f32r, kind="ExternalInput")
    outT = nc.dram_tensor("outT", [C, L], f32, kind="ExternalOutput")

    NLT = L // 128  # 8 l-tiles
    NCC = C // 128  # 4 feature chunks
    NH = L // 512   # 2 free-dim halves
    KQ = 64 + 12    # 76 contraction rows for the logits matmul

    with tile.TileContext(nc) as tc:
        with (
            tc.tile_pool(name="const", bufs=1) as const,
            tc.tile_pool(name="xin", bufs=3) as xin,
            tc.tile_pool(name="stats", bufs=4) as stats,
            tc.tile_pool(name="xn", bufs=8) as xnp,
            tc.tile_pool(name="xt", bufs=1) as xtp,
            tc.tile_pool(name="heads", bufs=2) as heads,
            tc.tile_pool(name="attn", bufs=4) as attnp,
            tc.tile_pool(name="ps", bufs=4, space="PSUM") as psp,
            tc.tile_pool(name="po", bufs=4, space="PSUM") as pop,
        ):
            def mmr(out, lhsT, rhs, **kw):
                nc.tensor.matmul(out, lhsT=lhsT.bitcast(f32r),
                                 rhs=rhs.bitcast(f32r), **kw)

            def tpr(out, in_, identity):
                nc.tensor.transpose(out.bitcast(f32r), in_.bitcast(f32r),
                                    identity.bitcast(f32r))

            # ---- constants / weights ----
            # Memset/affine_select cannot emit float32r, so constants are built
            # in f32 scratch and copied (DVE rounds on write).
            ident_sc = const.tile([128, 128], f32)
            make_identity(nc, ident_sc)
            ident = const.tile([128, 128], f32r)
            nc.vector.tensor_copy(ident, ident_sc)
            ones_sc = const.tile([128, L], f32)
            nc.gpsimd.memset(ones_sc, 1.0)
            eps_t = const.tile([128, 1], f32)
            nc.gpsimd.memset(eps_t, LN_EPS)
            ones64 = const.tile([1, D], f32r)
            nc.vector.tensor_copy(ones64, ones_sc[0:1, 0:D])
            if use_bo:
                ones_q = const.tile([1, L], f32r)
                nc.vector.tensor_copy(ones_q, ones_sc[0:1, :])

            wq_sb = const.tile([128, NCC, HPC * D], f32r)
            wk_sb = const.tile([128, NCC, HPC * D], f32r)
            wv_sb = const.tile([128, NCC, HPC * D], f32r)
            wpt_sb = const.tile([128, NCC, 128], f32r)
            for cc in range(NCC):
                nc.sync.dma_start(out=wq_sb[:, cc, :], in_=wq2[cc * 128:(cc + 1) * 128, :])
                nc.sync.dma_start(out=wk_sb[:, cc, :], in_=wk2[cc * 128:(cc + 1) * 128, :])
                nc.sync.dma_start(out=wv_sb[:, cc, :], in_=wv2[cc * 128:(cc + 1) * 128, :])
                nc.sync.dma_start(out=wpt_sb[:, cc, :], in_=wpt[cc * 128:(cc + 1) * 128, :])
            wo_sb = const.tile([D, HPC, C], f32r)
            for h in range(HPC):
                nc.sync.dma_start(out=wo_sb[:, h, :], in_=wo2[h * D:(h + 1) * D, :])
            pscpt_sb = const.tile([128, 1], f32)
            nc.sync.dma_start(out=pscpt_sb, in_=psc_pt[:, :])
            pscneg_sb = const.tile([128, HPC], f32)
            nc.sync.dma_start(out=pscneg_sb, in_=psc_negcol[:, :])
            if use_bias:
                bq_sb = const.tile([HPC * D, 1], f32)
                bk_sb = const.tile([HPC * D, 1], f32)
                bv_sb = const.tile([HPC * D, 1], f32)
                bpt_sb = const.tile([128, 1], f32)
                nc.sync.dma_start(out=bq_sb, in_=bq2[:, :])
                nc.sync.dma_start(out=bk_sb, in_=bk2[:, :])
                nc.sync.dma_start(out=bv_sb, in_=bv2[:, :])
                nc.sync.dma_start(out=bpt_sb, in_=b_pt[:, :])
            if use_bo:
                bo_sb = const.tile([1, C], f32r)
                nc.sync.dma_start(out=bo_sb, in_=bo_d[:, :])

            # ---- Phase A: load + layernorm (natural [l, c] layout) ----
            xn_tiles = []
            for lt in range(NLT):
                x_t = xin.tile([128, C], f32, tag="x")
                nc.sync.dma_start(out=x_t, in_=xb[lt * 128:(lt + 1) * 128, :])
                st = stats.tile([128, 6], f32, tag="st")
                nc.vector.bn_stats(out=st, in_=x_t)
                mv = stats.tile([128, 2], f32, tag="mv")
                nc.vector.bn_aggr(out=mv, in_=st)
                sd = stats.tile([128, 1], f32, tag="sd")
                nc.scalar.activation(out=sd, in_=mv[:, 1:2], func=AF.Sqrt, bias=eps_t)
                rstd = stats.tile([128, 1], f32, tag="rstd")
                nc.vector.reciprocal(out=rstd, in_=sd)
                xn_t = xnp.tile([128, C], f32r, tag="xn")
                nc.vector.tensor_scalar(
                    out=xn_t, in0=x_t, scalar1=mv[:, 0:1], scalar2=rstd,
                    op0=ALU.subtract, op1=ALU.mult)
                xn_tiles.append(xn_t)

            # ---- Phase B: transpose to feature-major xT [c, l] ----
            xT = xtp.tile([128, NCC, L], f32r)
            for half in range(NH):
                for cc in range(NCC):
                    ps = psp.tile([128, 512], f32, tag="ps")
                    for i in range(4):
                        lt = half * 4 + i
                        tpr(ps[:, i * 128:(i + 1) * 128],
                            xn_tiles[lt][:, cc * 128:(cc + 1) * 128], ident)
                    nc.vector.tensor_copy(xT[:, cc, half * 512:(half + 1) * 512], ps)

            # ---- Phase C: projections + per-head Q~/K~ assembly ----
            qt_t = [heads.tile([KQ, L], f32r, tag="qt", name=f"qt{h}") for h in range(HPC)]
            kt_t = [heads.tile([KQ, L], f32r, tag="kt", name=f"kt{h}") for h in range(HPC)]
            vT2 = heads.tile([128, L], f32r, tag="vt2")

            for half in range(NH):
                sl = slice(half * 512, (half + 1) * 512)
                # scalar q/k/v (both heads fused: M=128)
                for (w_sb, b_name) in ((wq_sb, "q"), (wk_sb, "k"), (wv_sb, "v")):
                    psm = psp.tile([128, 512], f32, tag="ps")
                    for cc in range(NCC):
                        mmr(psm, wq_sb[:, cc, :] if b_name == "q" else
                            (wk_sb[:, cc, :] if b_name == "k" else wv_sb[:, cc, :]),
                            xT[:, cc, sl],
                            start=(cc == 0), stop=(cc == NCC - 1))
                    if b_name == "q":
                        for h in range(HPC):
                            nc.scalar.activation(
                                out=qt_t[h][0:D, sl], in_=psm[h * D:(h + 1) * D, :],
                                func=AF.Identity, scale=float(D) ** -0.5,
                                bias=(bq_sb[h * D:(h + 1) * D, :] if use_bias else 0.0))
                    elif b_name == "k":
                        for h in range(HPC):
                            nc.scalar.activation(
                                out=kt_t[h][0:D, sl], in_=psm[h * D:(h + 1) * D, :],
                                func=AF.Identity,
                                bias=(bk_sb[h * D:(h + 1) * D, :] if use_bias else 0.0))
                    else:
                        if use_bias:
                            nc.vector.tensor_scalar(
                                out=vT2[:, sl], in0=psm, scalar1=bv_sb,
                                scalar2=None, op0=ALU.add)
                        else:
                            nc.vector.tensor_copy(vT2[:, sl], psm)
                # fused point q/k, heads padded to 32-partition blocks:
                # rows [0:12] q_p h0, [32:44] q_p h1, [64:76] k_p h0, [96:108] k_p h1
                pspt = psp.tile([128, 512], f32, tag="ps")
                for cc in range(NCC):
                    mmr(pspt, wpt_sb[:, cc, :], xT[:, cc, sl],
                        start=(cc == 0), stop=(cc == NCC - 1))
                for h in range(HPC):
                    qb, kb = h * 32, 64 + h * 32
                    nc.vector.tensor_scalar(
                        out=qt_t[h][D:D + 12, sl], in0=pspt[qb:qb + 12, :],
                        scalar1=(bpt_sb[qb:qb + 12, :] if use_bias else 0.0),
                        scalar2=pscpt_sb[qb:qb + 12, :],
                        op0=ALU.add, op1=ALU.mult)
                    nc.vector.tensor_scalar(
                        out=kt_t[h][D:D + 12, sl], in0=pspt[kb:kb + 12, :],
                        scalar1=(bpt_sb[kb:kb + 12, :] if use_bias else 0.0),
                        scalar2=pscpt_sb[kb:kb + 12, :],
                        op0=ALU.add, op1=ALU.mult)

            # ksb[h][:, jt] = -pscale[h] * |k_p|^2 for k-block jt (128 rows)
            ksb = []
            for h in range(HPC):
                kb_t = heads.tile([128, NLT], f32, tag="ksb", name=f"ksb{h}")
                for jt in range(NLT):
                    pst2 = psp.tile([128, 12], f32, tag="ps", name=f"pkp{h}_{jt}")
                    tpr(pst2, kt_t[h][D:D + 12, jt * 128:(jt + 1) * 128],
                        ident[64:76, 64:76])
                    sq = stats.tile([128, 12], f32, tag="sq")
                    kcol = stats.tile([128, 1], f32, tag="kcol")
                    nc.scalar.activation(out=sq, in_=pst2, func=AF.Square,
                                         accum_out=kcol)
                    nc.vector.tensor_scalar(
                        out=kb_t[:, jt:jt + 1], in0=kcol,
                        scalar1=pscneg_sb[:, h:h + 1], scalar2=None, op0=ALU.mult)
                ksb.append(kb_t)

            # V natural layout per head, with appended ones column (row-sum trick)
            vaug = []
            for h in range(HPC):
                va = heads.tile([128, NLT, D + 1], f32r, tag="vaug", name=f"va{h}")
                for jt in range(NLT):
                    pst = psp.tile([128, 512], f32, tag="ps", name=f"pvt{h}_{jt}")
                    tpr(pst[:, 0:D],
                        vT2[h * D:(h + 1) * D, jt * 128:(jt + 1) * 128],
                        ident[h * D:(h + 1) * D, h * D:(h + 1) * D])
                    nc.vector.tensor_copy(va[:, jt, 0:D], pst[:, 0:D])
                nc.vector.tensor_copy(va[:, :, D:D + 1], ones_sc[:, 0:NLT])
                vaug.append(va)

            # ---- Phase D: attention per head (transposed logits) ----
            onorm = []
            for h in range(HPC):
                po = [pop.tile([D + 1, 512], f32, tag="po", name=f"po{h}_{i}")
                      for i in range(NH)]
                for jt in range(NLT):
                    at_t = attnp.tile([128, L], f32r, tag="at")
                    for half in range(NH):
                        sl = slice(half * 512, (half + 1) * 512)
                        pss = psp.tile([128, 512], f32, tag="ps")
                        mmr(pss, kt_t[h][:, jt * 128:(jt + 1) * 128], qt_t[h][:, sl])
                        nc.scalar.activation(out=at_t[:, sl], in_=pss, func=AF.Exp,
                                             bias=ksb[h][:, jt:jt + 1])
                    for half in range(NH):
                        sl = slice(half * 512, (half + 1) * 512)
                        mmr(po[half], vaug[h][:, jt, :], at_t[:, sl],
                            start=(jt == 0), stop=(jt == NLT - 1))
                # normalize: out[d,i] * (1/sum[i]), broadcast via rank-1 matmul
                onm = heads.tile([D, L], f32r, tag="onorm", name=f"on{h}")
                recip = heads.tile([1, L], f32r, tag="recip", name=f"rc{h}")
                for half in range(NH):
                    sl = slice(half * 512, (half + 1) * 512)
                    with nc.allow_low_precision(reason="1/softmax-sum fits f32r"):
                        nc.vector.reciprocal(out=recip[0:1, sl],
                                             in_=po[half][D:D + 1, :])
                    psb = psp.tile([D, 512], f32, tag="ps")
                    mmr(psb, ones64, recip[0:1, sl])
                    osc = heads.tile([D, 512], f32, tag="osc", name=f"osc{h}_{half}")
                    nc.scalar.activation(out=osc, in_=po[half][0:D, :], func=AF.Copy)
                    nc.vector.tensor_mul(onm[0:D, sl], osc, psb)
                onorm.append(onm)

            # ---- Phase E: output projection (row-sharded wo) ----
            for cc in range(NCC):
                for half in range(NH):
                    sl = slice(half * 512, (half + 1) * 512)
                    csl = slice(cc * 128, (cc + 1) * 128)
                    psw = psp.tile([128, 512], f32, tag="ps")
                    mmr(psw, wo_sb[:, 0, csl], onorm[0][0:D, sl],
                        start=True, stop=False)
                    mmr(psw, wo_sb[:, 1, csl], onorm[1][0:D, sl],
                        start=False, stop=not use_bo)
                    if use_bo:
                        mmr(psw, bo_sb[0:1, csl], ones_q[0:1, sl],
                            start=False, stop=True)
                    ow = attnp.tile([128, 512], f32, tag="ow", name=f"ow{cc}_{half}")
                    if (cc + half) % 2 == 0:
                        nc.vector.tensor_copy(ow, psw)
                    else:
                        nc.scalar.activation(out=ow, in_=psw, func=AF.Copy)
                    nc.sync.dma_start(out=outT[csl, sl], in_=ow)

    nc.finalize()
    return nc


def _prepare(inputs):
    """Returns (use_bias, use_bo, in_maps) from full unsharded inputs."""
    features = np.ascontiguousarray(np.asarray(inputs["features"], dtype=np.float32))
    ln_g = np.asarray(inputs["ln_g"], dtype=np.float32)
    ln_b = np.asarray(inputs["ln_b"], dtype=np.float32)
    wq = np.asarray(inputs["wq"], dtype=np.float32)
    wk = np.asarray(inputs["wk"], dtype=np.float32)
    wv = np.asarray(inputs["wv"], dtype=np.float32)
    wqp = np.asarray(inputs["wqp"], dtype=np.float32)
    wkp = np.asarray(inputs["wkp"], dtype=np.float32)
    wo = np.asarray(inputs["wo"], dtype=np.float32)
    bq = np.asarray(inputs["bq"], dtype=np.float32)
    bk = np.asarray(inputs["bk"], dtype=np.float32)
    bv = np.asarray(inputs["bv"], dtype=np.float32)
    bqp = np.asarray(inputs["bqp"], dtype=np.float32)
    bkp = np.asarray(inputs["bkp"], dtype=np.float32)
    bo = np.asarray(inputs["bo"], dtype=np.float32)
    pscale = np.asarray(inputs["pscale"], dtype=np.float32)

    # Fold ln_g into projection weights; ln_b + proj bias into per-channel consts.
    g = ln_g[:, None]
    wq_f, wk_f, wv_f = wq * g, wk * g, wv * g
    wqp_f, wkp_f = wqp * g, wkp * g
    bq_f = ln_b @ wq + bq
    bk_f = ln_b @ wk + bk
    bv_f = ln_b @ wv + bv
    bqp_f = ln_b @ wqp + bqp
    bkp_f = ln_b @ wkp + bkp

    use_bias = bool(np.any(bq_f) or np.any(bk_f) or np.any(bv_f)
                    or np.any(bqp_f) or np.any(bkp_f))
    use_bo = bool(np.any(bo))

    in_maps = []
    for c in range(NCORES):
        b, hg = divmod(c, 4)
        h0 = HPC * hg
        dsl = slice(h0 * D, (h0 + HPC) * D)
        ps2 = pscale[h0:h0 + HPC]

        wpt_m = np.zeros((C, 128), np.float32)
        b_pt_m = np.zeros((128, 1), np.float32)
        psc_pt_m = np.zeros((128, 1), np.float32)
        for h in range(HPC):
            psl = slice((h0 + h) * 12, (h0 + h + 1) * 12)
            wpt_m[:, h * 32:h * 32 + 12] = wqp_f[:, psl]
            wpt_m[:, 64 + h * 32:64 + h * 32 + 12] = wkp_f[:, psl]
            b_pt_m[h * 32:h * 32 + 12, 0] = bqp_f[psl]
            b_pt_m[64 + h * 32:64 + h * 32 + 12, 0] = bkp_f[psl]
            psc_pt_m[h * 32:h * 32 + 12, 0] = 2.0 * ps2[h]
            psc_pt_m[64 + h * 32:64 + h * 32 + 12, 0] = 1.0

        m = {
            "xb": np.ascontiguousarray(features[b]),
            "wq2": np.ascontiguousarray(wq_f[:, dsl]),
            "wk2": np.ascontiguousarray(wk_f[:, dsl]),
            "wv2": np.ascontiguousarray(wv_f[:, dsl]),
            "wpt": wpt_m,
            "wo2": np.ascontiguousarray(wo[dsl, :]),
            "psc_pt": psc_pt_m,
            "psc_negcol": np.tile(-ps2[None, :], (128, 1)).astype(np.float32),
        }
        if use_bias:
            m["bq2"] = np.ascontiguousarray(bq_f[dsl])[:, None]
            m["bk2"] = np.ascontiguousarray(bk_f[dsl])[:, None]
            m["bv2"] = np.ascontiguousarray(bv_f[dsl])[:, None]
            m["b_pt"] = b_pt_m
        if use_bo:
            m["bo_d"] = (bo if hg == 0 else np.zeros_like(bo))[None, :]
        in_maps.append(m)
    return use_bias, use_bo, in_maps


def kernel(**inputs):
    from concourse.bass_utils import run_bass_kernel_spmd

    use_bias, use_bo, in_maps = _prepare(inputs)
    key = (use_bias, use_bo)
    if key not in _CACHE:
        _CACHE[key] = _build_program(use_bias, use_bo)
    nc = _CACHE[key]

    res = run_bass_kernel_spmd(nc, in_maps, list(range(NCORES)))

    out = np.zeros((B, L, C), dtype=np.float32)
    for c in range(NCORES):
        out[c // 4] += res.results[c]["outT"].T
    return out


# revision 35
# speedup vs baseline: 1.1862x; 1.1862x over previous
"""Invariant Point Attention kernel for 8 Trainium2 NeuronCores.

Shapes (hardcoded): features [2, 1024, 512], H=8 heads, D=64, P=4 points.
Sharding: 8 cores = 2 batches x 4 head-pairs. Core c handles batch c//4 and
heads (2*(c%4), 2*(c%4)+1). Each core computes its heads' attention plus the
row-slice of the output projection; the host sums the 4 partials per batch.

Math notes:
  - softmax(logits) is invariant to per-row shifts, so the -pscale*|q_p|^2 term
    is dropped entirely; the -pscale*|k_p|^2 term enters as a per-partition
    bias on the Exp activation (logits are computed transposed, so k-index j
    is the partition dim there).
  - ln_g folds into the projection weights on the host; ln_b and projection
    biases fold into per-output-channel constants.
  - The softmax denominator comes from an appended ones-column in the V matrix
    of the attention*V matmul (row 64 of the accumulator = row sums).
  - Matmul operands are bitcast to float32r (full-rate fp32 PE mode).
  - Point q/k projections for both heads are fused into one M=128 stationary
    matrix, each 12-wide block padded to a 32-partition boundary so PSUM reads
    stay quadrant-aligned.
"""

import numpy as np

B, L, C = 2, 1024, 512
H, P = 8, 4
D = C // H  # 64
LN_EPS = 1e-5
NCORES = 8
HPC = 2  # heads per core

_CACHE = {}


def _build_program(use_bias: bool, use_bo: bool):
    import concourse.bacc as bacc
    import concourse.tile as tile
    import concourse.mybir as mybir
    from concourse.masks import make_identity

    f32 = mybir.dt.float32
    f32r = mybir.dt.float32r
    AF = mybir.ActivationFunctionType
    ALU = mybir.AluOpType

    nc = bacc.Bacc(None, target_bir_lowering=False)

    # ---- DRAM I/O (per-core contents supplied via in_maps) ----
    xb = nc.dram_tensor("xb", [L, C], f32, kind="ExternalInput")
    # packed [q | k | v | pt] projection weights: [C, 4, 128]
    wcat = nc.dram_tensor("wcat", [C, 4, 128], f32r, kind="ExternalInput")
    wo2 = nc.dram_tensor("wo2", [HPC * D, C], f32r, kind="ExternalInput")
    # packed per-partition scales: col0 = point scale, col1:3 = -pscale
    psc_cat = nc.dram_tensor("psc_cat", [128, 1 + HPC], f32, kind="ExternalInput")
    if use_bias:
        bq2 = nc.dram_tensor("bq2", [HPC * D, 1], f32, kind="ExternalInput")
        bk2 = nc.dram_tensor("bk2", [HPC * D, 1], f32, kind="ExternalInput")
        bv2 = nc.dram_tensor("bv2", [HPC * D, 1], f32, kind="ExternalInput")
        b_pt = nc.dram_tensor("b_pt", [128, 1], f32, kind="ExternalInput")
    if use_bo:
        bo_d = nc.dram_tensor("bo_d", [1, C], f32r, kind="ExternalInput")
    outT = nc.dram_tensor("outT", [C, L], f32, kind="ExternalOutput")

    NLT = L // 128  # 8 l-tiles
    NCC = C // 128  # 4 feature chunks
    NH = L // 512   # 2 free-dim halves
    KQ = 64 + 12    # 76 contraction rows for the logits matmul

    with tile.TileContext(nc) as tc:
        with (
            tc.tile_pool(name="const", bufs=1) as const,
            tc.tile_pool(name="xin", bufs=8) as xin,
            tc.tile_pool(name="stats", bufs=4) as stats,
            tc.tile_pool(name="xn", bufs=8) as xnp,
            tc.tile_pool(name="xt", bufs=1) as xtp,
            tc.tile_pool(name="heads", bufs=2) as heads,
            tc.tile_pool(name="attn", bufs=4) as attnp,
            tc.tile_pool(name="ps", bufs=4, space="PSUM") as psp,
            tc.tile_pool(name="po", bufs=4, space="PSUM") as pop,
        ):
            def mmr(out, lhsT, rhs, **kw):
                nc.tensor.matmul(out, lhsT=lhsT.bitcast(f32r),
                                 rhs=rhs.bitcast(f32r), **kw)

            def tpr(out, in_, identity):
                nc.tensor.transpose(out.bitcast(f32r), in_.bitcast(f32r),
                                    identity.bitcast(f32r))

            # ---- constants / weights ----
            # Memset/affine_select cannot emit float32r, so constants are built
            # in f32 scratch and copied (DVE rounds on write).
            ident_sc = const.tile([128, 128], f32)
            make_identity(nc, ident_sc)
            ident = const.tile([128, 128], f32r)
            nc.vector.tensor_copy(ident, ident_sc)
            ones_sc = const.tile([128, L], f32)
            nc.gpsimd.memset(ones_sc, 1.0)
            eps_t = const.tile([128, 1], f32)
            nc.gpsimd.memset(eps_t, LN_EPS)
            ones64 = const.tile([1, D], f32r)
            nc.vector.tensor_copy(ones64, ones_sc[0:1, 0:D])
            if use_bo:
                ones_q = const.tile([1, L], f32r)
                nc.vector.tensor_copy(ones_q, ones_sc[0:1, :])

            w_all = const.tile([128, NCC, 4, 128], f32r)
            for cc in range(NCC):
                nc.scalar.dma_start(out=w_all[:, cc, :, :],
                                    in_=wcat[cc * 128:(cc + 1) * 128, :, :])
            wo_sb = const.tile([D, HPC, C], f32r)
            for h in range(HPC):
                nc.scalar.dma_start(out=wo_sb[:, h, :], in_=wo2[h * D:(h + 1) * D, :])
            psc_sb = const.tile([128, 1 + HPC], f32)
            nc.scalar.dma_start(out=psc_sb, in_=psc_cat[:, :])
            pscpt_sb = psc_sb[:, 0:1]
            pscneg_sb = psc_sb[:, 1:1 + HPC]
            if use_bias:
                bq_sb = const.tile([HPC * D, 1], f32)
                bk_sb = const.tile([HPC * D, 1], f32)
                bv_sb = const.tile([HPC * D, 1], f32)
                bpt_sb = const.tile([128, 1], f32)
                nc.scalar.dma_start(out=bq_sb, in_=bq2[:, :])
                nc.scalar.dma_start(out=bk_sb, in_=bk2[:, :])
                nc.scalar.dma_start(out=bv_sb, in_=bv2[:, :])
                nc.scalar.dma_start(out=bpt_sb, in_=b_pt[:, :])
            if use_bo:
                bo_sb = const.tile([1, C], f32r)
                nc.scalar.dma_start(out=bo_sb, in_=bo_d[:, :])

            # ---- Phase A: load + layernorm (natural [l, c] layout) ----
            xn_tiles = []
            x_tiles = []
            for lt in range(NLT):
                x_t = xin.tile([128, C], f32, tag="x", name=f"x{lt}")
                nc.sync.dma_start(out=x_t, in_=xb[lt * 128:(lt + 1) * 128, :])
                x_tiles.append(x_t)
            for lt in range(NLT):
                x_t = x_tiles[lt]
                st = stats.tile([128, 6], f32, tag="st")
                nc.vector.bn_stats(out=st, in_=x_t)
                mv = stats.tile([128, 2], f32, tag="mv")
                nc.vector.bn_aggr(out=mv, in_=st)
                sd = stats.tile([128, 1], f32, tag="sd")
                nc.scalar.activation(out=sd, in_=mv[:, 1:2], func=AF.Sqrt, bias=eps_t)
                rstd = stats.tile([128, 1], f32, tag="rstd")
                nc.vector.reciprocal(out=rstd, in_=sd)
                xn_t = xnp.tile([128, C], f32r, tag="xn")
                nc.vector.tensor_scalar(
                    out=xn_t, in0=x_t, scalar1=mv[:, 0:1], scalar2=rstd,
                    op0=ALU.subtract, op1=ALU.mult)
                xn_tiles.append(xn_t)

            # ---- Phase B: transpose to feature-major xT [c, l] ----
            xT = xtp.tile([128, NCC, L], f32r)
            for half in range(NH):
                for cc in range(NCC):
                    ps = psp.tile([128, 512], f32, tag="ps")
                    for i in range(4):
                        lt = half * 4 + i
                        tpr(ps[:, i * 128:(i + 1) * 128],
                            xn_tiles[lt][:, cc * 128:(cc + 1) * 128], ident)
                    nc.vector.tensor_copy(xT[:, cc, half * 512:(half + 1) * 512], ps)

            # ---- Phase C: projections + per-head Q~/K~ assembly ----
            qt_t = [heads.tile([KQ, L], f32r, tag="qt", name=f"qt{h}") for h in range(HPC)]
            kt_t = [heads.tile([KQ, L], f32r, tag="kt", name=f"kt{h}") for h in range(HPC)]
            vT2 = heads.tile([128, L], f32r, tag="vt2")

            for half in range(NH):
                sl = slice(half * 512, (half + 1) * 512)
                # scalar q/k/v (both heads fused: M=128)
                for gi, b_name in ((0, "q"), (1, "k"), (2, "v")):
                    psm = psp.tile([128, 512], f32, tag="ps")
                    for cc in range(NCC):
                        mmr(psm, w_all[:, cc, gi, :], xT[:, cc, sl],
                            start=(cc == 0), stop=(cc == NCC - 1))
                    if b_name == "q":
                        for h in range(HPC):
                            nc.scalar.activation(
                                out=qt_t[h][0:D, sl], in_=psm[h * D:(h + 1) * D, :],
                                func=AF.Identity, scale=float(D) ** -0.5,
                                bias=(bq_sb[h * D:(h + 1) * D, :] if use_bias else 0.0))
                    elif b_name == "k":
                        for h in range(HPC):
                            nc.scalar.activation(
                                out=kt_t[h][0:D, sl], in_=psm[h * D:(h + 1) * D, :],
                                func=AF.Identity,
                                bias=(bk_sb[h * D:(h + 1) * D, :] if use_bias else 0.0))
                    else:
                        if use_bias:
                            nc.vector.tensor_scalar(
                                out=vT2[:, sl], in0=psm, scalar1=bv_sb,
                                scalar2=None, op0=ALU.add)
                        else:
                            nc.vector.tensor_copy(vT2[:, sl], psm)
                # fused point q/k, heads padded to 32-partition blocks:
                # rows [0:12] q_p h0, [32:44] q_p h1, [64:76] k_p h0, [96:108] k_p h1
                pspt = psp.tile([128, 512], f32, tag="ps")
                for cc in range(NCC):
                    mmr(pspt, w_all[:, cc, 3, :], xT[:, cc, sl],
                        start=(cc == 0), stop=(cc == NCC - 1))
                for h in range(HPC):
                    qb, kb = h * 32, 64 + h * 32
                    nc.vector.tensor_scalar(
                        out=qt_t[h][D:D + 12, sl], in0=pspt[qb:qb + 12, :],
                        scalar1=(bpt_sb[qb:qb + 12, :] if use_bias else 0.0),
                        scalar2=pscpt_sb[qb:qb + 12, :],
                        op0=ALU.add, op1=ALU.mult)
                    nc.vector.tensor_scalar(
                        out=kt_t[h][D:D + 12, sl], in0=pspt[kb:kb + 12, :],
                        scalar1=(bpt_sb[kb:kb + 12, :] if use_bias else 0.0),
                        scalar2=pscpt_sb[kb:kb + 12, :],
                        op0=ALU.add, op1=ALU.mult)

            # ksb[h][:, jt] = -pscale[h] * |k_p|^2 for k-block jt (128 rows)
            ksb = []
            for h in range(HPC):
                kb_t = heads.tile([128, NLT], f32, tag="ksb", name=f"ksb{h}")
                for jt in range(NLT):
                    pst2 = psp.tile([128, 12], f32, tag="ps", name=f"pkp{h}_{jt}")
                    tpr(pst2, kt_t[h][D:D + 12, jt * 128:(jt + 1) * 128],
                        ident[64:76, 64:76])
                    sq = stats.tile([128, 12], f32, tag="sq")
                    kcol = stats.tile([128, 1], f32, tag="kcol")
                    nc.scalar.activation(out=sq, in_=pst2, func=AF.Square,
                                         accum_out=kcol)
                    nc.vector.tensor_scalar(
                        out=kb_t[:, jt:jt + 1], in0=kcol,
                        scalar1=pscneg_sb[:, h:h + 1], scalar2=None, op0=ALU.mult)
                ksb.append(kb_t)

            # V natural layout per head, with appended ones column (row-sum trick)
            vaug = []
            for h in range(HPC):
                va = heads.tile([128, NLT, D + 1], f32r, tag="vaug", name=f"va{h}")
                for jt in range(NLT):
                    pst = psp.tile([128, 512], f32, tag="ps", name=f"pvt{h}_{jt}")
                    tpr(pst[:, 0:D],
                        vT2[h * D:(h + 1) * D, jt * 128:(jt + 1) * 128],
                        ident[h * D:(h + 1) * D, h * D:(h + 1) * D])
                    nc.vector.tensor_copy(va[:, jt, 0:D], pst[:, 0:D])
                nc.vector.tensor_copy(va[:, :, D:D + 1], ones_sc[:, 0:NLT])
                vaug.append(va)

            # ---- Phase D: attention per head (transposed logits) ----
            onorm = []
            for h in range(HPC):
                po = [pop.tile([D + 1, 512], f32, tag="po", name=f"po{h}_{i}")
                      for i in range(NH)]
                for jt in range(NLT):
                    at_t = attnp.tile([128, L], f32r, tag="at")
                    for half in range(NH):
                        sl = slice(half * 512, (half + 1) * 512)
                        pss = psp.tile([128, 512], f32, tag="ps")
                        mmr(pss, kt_t[h][:, jt * 128:(jt + 1) * 128], qt_t[h][:, sl])
                        nc.scalar.activation(out=at_t[:, sl], in_=pss, func=AF.Exp,
                                             bias=ksb[h][:, jt:jt + 1])
                    for half in range(NH):
                        sl = slice(half * 512, (half + 1) * 512)
                        mmr(po[half], vaug[h][:, jt, :], at_t[:, sl],
                            start=(jt == 0), stop=(jt == NLT - 1))
                # normalize: out[d,i] * (1/sum[i]), broadcast via rank-1 matmul
                onm = heads.tile([D, L], f32r, tag="onorm", name=f"on{h}")
                recip = heads.tile([1, L], f32r, tag="recip", name=f"rc{h}")
                for half in range(NH):
                    sl = slice(half * 512, (half + 1) * 512)
                    with nc.allow_low_precision(reason="1/softmax-sum fits f32r"):
                        nc.vector.reciprocal(out=recip[0:1, sl],
                                             in_=po[half][D:D + 1, :])
                    psb = psp.tile([D, 512], f32, tag="ps")
                    mmr(psb, ones64, recip[0:1, sl])
                    osc = heads.tile([D, 512], f32, tag="osc", name=f"osc{h}_{half}")
                    nc.scalar.activation(out=osc, in_=po[half][0:D, :], func=AF.Copy)
                    nc.vector.tensor_mul(onm[0:D, sl], osc, psb)
                onorm.append(onm)

            # ---- Phase E: output projection (row-sharded wo) ----
            for cc in range(NCC):
                for half in range(NH):
                    sl = slice(half * 512, (half + 1) * 512)
                    csl = slice(cc * 128, (cc + 1) * 128)
                    psw = psp.tile([128, 512], f32, tag="ps")
                    mmr(psw, wo_sb[:, 0, csl], onorm[0][0:D, sl],
                        start=True, stop=False)
                    mmr(psw, wo_sb[:, 1, csl], onorm[1][0:D, sl],
                        start=False, stop=not use_bo)
                    if use_bo:
                        mmr(psw, bo_sb[0:1, csl], ones_q[0:1, sl],
                            start=False, stop=True)
                    ow = attnp.tile([128, 512], f32, tag="ow", name=f"ow{cc}_{half}")
                    if (cc + half) % 2 == 0:
                        nc.vector.tensor_copy(ow, psw)
                    else:
                        nc.scalar.activation(out=ow, in_=psw, func=AF.Copy)
                    nc.sync.dma_start(out=outT[csl, sl], in_=ow)

    nc.finalize()
    return nc


def _prepare(inputs):
    """Returns (use_bias, use_bo, in_maps) from full unsharded inputs."""
    features = np.ascontiguousarray(np.asarray(inputs["features"], dtype=np.float32))
    ln_g = np.asarray(inputs["ln_g"], dtype=np.float32)
    ln_b = np.asarray(inputs["ln_b"], dtype=np.float32)
    wq = np.asarray(inputs["wq"], dtype=np.float32)
    wk = np.asarray(inputs["wk"], dtype=np.float32)
    wv = np.asarray(inputs["wv"], dtype=np.float32)
    wqp = np.asarray(inputs["wqp"], dtype=np.float32)
    wkp = np.asarray(inputs["wkp"], dtype=np.float32)
    wo = np.asarray(inputs["wo"], dtype=np.float32)
    bq = np.asarray(inputs["bq"], dtype=np.float32)
    bk = np.asarray(inputs["bk"], dtype=np.float32)
    bv = np.asarray(inputs["bv"], dtype=np.float32)
    bqp = np.asarray(inputs["bqp"], dtype=np.float32)
    bkp = np.asarray(inputs["bkp"], dtype=np.float32)
    bo = np.asarray(inputs["bo"], dtype=np.float32)
    pscale = np.asarray(inputs["pscale"], dtype=np.float32)

    # Fold ln_g into projection weights; ln_b + proj bias into per-channel consts.
    g = ln_g[:, None]
    wq_f, wk_f, wv_f = wq * g, wk * g, wv * g
    wqp_f, wkp_f = wqp * g, wkp * g
    bq_f = ln_b @ wq + bq
    bk_f = ln_b @ wk + bk
    bv_f = ln_b @ wv + bv
    bqp_f = ln_b @ wqp + bqp
    bkp_f = ln_b @ wkp + bkp

    use_bias = bool(np.any(bq_f) or np.any(bk_f) or np.any(bv_f)
                    or np.any(bqp_f) or np.any(bkp_f))
    use_bo = bool(np.any(bo))

    in_maps = []
    for c in range(NCORES):
        b, hg = divmod(c, 4)
        h0 = HPC * hg
        dsl = slice(h0 * D, (h0 + HPC) * D)
        ps2 = pscale[h0:h0 + HPC]

        wpt_m = np.zeros((C, 128), np.float32)
        b_pt_m = np.zeros((128, 1), np.float32)
        psc_pt_m = np.zeros((128, 1), np.float32)
        for h in range(HPC):
            psl = slice((h0 + h) * 12, (h0 + h + 1) * 12)
            wpt_m[:, h * 32:h * 32 + 12] = wqp_f[:, psl]
            wpt_m[:, 64 + h * 32:64 + h * 32 + 12] = wkp_f[:, psl]
            b_pt_m[h * 32:h * 32 + 12, 0] = bqp_f[psl]
            b_pt_m[64 + h * 32:64 + h * 32 + 12, 0] = bkp_f[psl]
            psc_pt_m[h * 32:h * 32 + 12, 0] = 2.0 * ps2[h]
            psc_pt_m[64 + h * 32:64 + h * 32 + 12, 0] = 1.0

        wcat_m = np.stack([wq_f[:, dsl], wk_f[:, dsl], wv_f[:, dsl], wpt_m],
                          axis=1)  # [C, 4, 128]
        psc_cat_m = np.concatenate(
            [psc_pt_m, np.tile(-ps2[None, :], (128, 1))], axis=1).astype(np.float32)

        m = {
            "xb": np.ascontiguousarray(features[b]),
            "wcat": np.ascontiguousarray(wcat_m),
            "wo2": np.ascontiguousarray(wo[dsl, :]),
            "psc_cat": psc_cat_m,
        }
        if use_bias:
            m["bq2"] = np.ascontiguousarray(bq_f[dsl])[:, None]
            m["bk2"] = np.ascontiguousarray(bk_f[dsl])[:, None]
            m["bv2"] = np.ascontiguousarray(bv_f[dsl])[:, None]
            m["b_pt"] = b_pt_m
        if use_bo:
            m["bo_d"] = (bo if hg == 0 else np.zeros_like(bo))[None, :]
        in_maps.append(m)
    return use_bias, use_bo, in_maps


def kernel(**inputs):
    from concourse.bass_utils import run_bass_kernel_spmd

    use_bias, use_bo, in_maps = _prepare(inputs)
    key = (use_bias, use_bo)
    if key not in _CACHE:
        _CACHE[key] = _build_program(use_bias, use_bo)
    nc = _CACHE[key]

    res = run_bass_kernel_spmd(nc, in_maps, list(range(NCORES)))

    out = np.zeros((B, L, C), dtype=np.float32)
    for c in range(NCORES):
        out[c // 4] += res.results[c]["outT"].T
    return out
